# revision 30
# baseline (speedup 1.0000x reference)
"""Trainium2 Bass kernel for nn_GAT_7851200217746 (hierarchical GAT message passing).

Algorithm (aggregate-first GAT restructuring):
  For each GAT layer application on (x_self [G,F], x_neigh [G,E,F], W, a_s, a_n):
    w_s[f,h] = sum_d W[f,h*D+d] a_s[h,d];  w_n likewise
    e_s = x_self @ w_s;  e_n = x_neigh @ w_n
    alpha = softmax_E(leaky_relu(e_s + e_n))
    x_agg[g,h,:] = sum_e alpha[g,e,h] x_neigh[g,e,:]    (aggregate in INPUT space)
    out[g, h*D:(h+1)*D] = x_agg[g,h,:] @ W[:, h*D:(h+1)*D]

Perf structure (v2):
  - x2 streamed twice: row-major bf16 (aggregation contraction) and
    feature-major fp8e4m3 (e_n logits only; validated 0.7% rel err).
  - All matmuls bf16/fp8 (no fp32 on the PE), stationaries padded to 128
    columns so FWL fires (2x bf16 / 4x fp8 weight loads).
  - e_s terms computed batched (one mm per 80-128 groups) and broadcast
    to neighbor rows via indicator-product expander matmuls that
    accumulate directly into the logits PSUM (expander writes first with
    start=True, e_n matmuls accumulate after).
  - leaky_relu+exp on the Scalar (ACT) engine straight from PSUM.
  - One packed const DMA (gpsimd queue) + x2r on sync + x2t on scalar
    queues; all tiles 128-partition padded with zero-padding chosen so
    junk rows produce exactly 0 in albd (no NaN can propagate).

Sharding: pure data-parallel over batch (128 batches/core x 8 cores).
"""

import sys

sys.path.insert(0, "/opt/trn_rl_repo")

from contextlib import ExitStack

import ml_dtypes
import numpy as np

import concourse.bass as bass
import concourse.tile as tile
from concourse import bacc, mybir
import concourse.bass_utils as bass_utils

BF = mybir.dt.bfloat16
F32 = mybir.dt.float32
FP8 = mybir.dt.float8e4
AF = mybir.ActivationFunctionType

NCORES = 8
B, FEAT, HID, OUT, H = 1024, 128, 512, 256, 4
BC = B // NCORES              # 128 batches per core
G1 = BC * 10                  # 1280 level-1 groups (h1 rows)
R2 = G1 * 25                  # 32000 h2 rows
TR = 125                      # stage-A tile rows (5 groups of 25)
TPS = 32                      # tiles per superiter
NSUP = R2 // (TR * TPS)       # 8 superiters
SUPR = TR * TPS               # 4000 rows per superiter
SUPG = SUPR // 25             # 160 groups per superiter
X2TP = 4032                   # x2t cols per superiter incl zero pad
G1P = 1344                    # x1t padded cols (>= 1200+128)
TB = 80                       # stage-B/C tile rows (8 groups of 10)
NTB = G1 // TB                # 16 tiles
LEAKY = 0.2

# packed bf16 const/per-core "smalls" layout: name -> n_cols
SMALLS = [
    ("x1tp", G1P),            # h1^T feature-major, padded with zeros
    ("x0t", BC),              # h0^T feature-major
    ("x1r", NTB * FEAT),      # h1 row-major tiles [80 rows used, pad 0]
    ("w0s4", H), ("w0n4", H),
    ("w1s4", 4 * H), ("w1n4", 4 * H),     # [128, 4, H] k-chunked combos
    ("w0b", HID),                          # W0 bf16
    ("w1b", 4 * HID),                      # [128, 4, 512]
    ("wfcb", 4 * OUT),                     # [128, 4, 256]
    ("e5p", 5),               # row->group indicator, rows>=125 zero
    ("e5xp", 128),            # group-sum expander + identity pad
    ("L80p", 128),            # esam expander [g' mod 5 == r div 25]
    ("LBp", 128),             # stage-B/C expander [b mod 8 == r div 10]
    ("ind16", 16),            # [g' div 5 == t'] (rows>=80 zero)
    ("indB16", 16),           # [b div 8 == i]
    ("i128b", 128),           # identity
    ("e10p", 8),              # stage-B/C row->group indicator (rows>=80 zero)
    ("e10xp", 128),           # stage-B/C group-sum expander + identity pad
]
SOFF = {}
_off = 0
for _n, _c in SMALLS:
    SOFF[_n] = _off
    _off += _c
SCOLS = _off


def build_program(debug_out=False):
    nc = bacc.Bacc(
        "TRN2",
        target_bir_lowering=False,
        debug=False,
        enable_asserts=False,
        num_devices=NCORES,
    )

    x2r = nc.dram_tensor("x2r", [NSUP * TR, TPS * FEAT], BF,
                         kind="ExternalInput").ap()
    x2t8 = nc.dram_tensor("x2t8", [FEAT, NSUP * X2TP], FP8,
                          kind="ExternalInput").ap()
    smalls = nc.dram_tensor("smalls", [128, SCOLS], BF,
                            kind="ExternalInput").ap()
    out_d = nc.dram_tensor("out", [BC, OUT], F32, kind="ExternalOutput").ap()

    with tile.TileContext(nc) as tc, ExitStack() as ctx:
        const = ctx.enter_context(tc.tile_pool(name="const", bufs=1))
        perst = ctx.enter_context(tc.tile_pool(name="perst", bufs=1))
        stream = ctx.enter_context(tc.tile_pool(name="stream", bufs=4))
        sm = ctx.enter_context(tc.tile_pool(name="sm", bufs=2))
        smb = ctx.enter_context(tc.tile_pool(name="smb", bufs=2))
        # PSUM budget is 8 bank-slots (2KB each). Layout:
        #  psA "en" x2   — logits; the softmax-sum mm reuses the same tile
        #  psC "cen"/"cagg" x1 — stage-C logits (+sum reuse) and agg
        #  psb "agg"/"hn" x2  — x_agg + h1_new/transpose/epilogue (2KB)
        psA = ctx.enter_context(tc.tile_pool(name="psA", bufs=2, space="PSUM"))
        psC = ctx.enter_context(tc.tile_pool(name="psC", bufs=1, space="PSUM"))
        psb = ctx.enter_context(tc.tile_pool(name="psb", bufs=2, space="PSUM"))

        sm_s = const.tile([128, SCOLS], BF, name="smalls")
        nc.gpsimd.dma_start(sm_s[:], smalls)

        def sv(name, split=None):
            """Slice view of the packed smalls tile."""
            c = dict(SMALLS)[name]
            v = sm_s[:, SOFF[name]:SOFF[name] + c]
            if split is not None:
                v = v.rearrange("p (a b) -> p a b", a=split)
            return v

        x1tp = sv("x1tp")
        x0t = sv("x0t")
        x1r = sv("x1r", NTB)
        w0s4 = sv("w0s4")
        w0n4 = sv("w0n4")
        w1s4 = sv("w1s4", 4)
        w1n4 = sv("w1n4", 4)
        w0b = sv("w0b")
        w1b = sv("w1b", 4)
        wfcb = sv("wfcb", 4)
        e5p = sv("e5p")
        e5xp = sv("e5xp")
        L80p = sv("L80p")
        LBp = sv("LBp")
        ind16 = sv("ind16")
        indB16 = sv("indB16")
        i128b = sv("i128b")
        e10p = sv("e10p")
        e10xp = sv("e10xp")

        h1t_s = perst.tile([128, H, G1P], BF)       # h1_new^T feature-major
        # zero h1t once: stage-C stationaries are padded to 128 columns and
        # read 48 columns ahead of what this superiter wrote — stale SBUF
        # there could be inf/NaN, and 0*NaN in the PE poisons the softmax.
        nc.gpsimd.memset(h1t_s[:], 0.0)
        h1r_s = perst.tile([TB, NTB, H, 128], BF)   # h1_new row-major
        xc_sb = perst.tile([128, 4, NTB, 8, H], BF)  # stage-C agg (d-major)
        R_all = perst.tile([128, 16, 16, H], BF)    # stage-A e_s expander rhs
        RB = perst.tile([128, 16, H], BF)           # stage-B e_s expander rhs
        RC = perst.tile([128, 16, H], BF)           # stage-C e_s expander rhs
        h0t_bf = perst.tile([128, H, BC], BF)       # stage-B output^T

        mm = nc.tensor.matmul

        # ---------- upfront: stage-A e_s for all superiters ----------
        # es80 chunk c: groups [80c, 80c+80).  R_all[g',c,t',h] =
        # e_s[80c+g', h] * [g' div 5 == t']
        es80_sb = smb.tile([128, 16, H], BF, tag="es80")
        for half in range(2):
            es_ps = psb.tile([128, 8, H], F32, tag="agg", name=f"es_ps{half}")
            for c_ in range(8):
                c = 8 * half + c_
                mm(es_ps[:, c_, :], x1tp[:, 80 * c:80 * c + 128], w0s4,
                   start=True, stop=True, skip_group_check=True)
            nc.vector.tensor_copy(es80_sb[:, 8 * half:8 * (half + 1), :],
                                  es_ps[:])
        nc.vector.tensor_mul(
            R_all[:],
            es80_sb[:].unsqueeze(2).broadcast_to((128, 16, 16, H)),
            ind16.unsqueeze(1).unsqueeze(3).broadcast_to((128, 16, 16, H)),
        )

        # ---------- STAGE B: layer0 on (h0 self, h1 neigh, E=10) ----------
        # esB = h0 @ w0s (one mm), RB = esB * indB16
        esB_ps = psb.tile([128, H], F32, tag="agg", name="esB_ps")
        mm(esB_ps[:], x0t, w0s4, start=True, stop=True, skip_group_check=True)
        esB_sb = smb.tile([128, H], BF, tag="esB")
        nc.vector.tensor_copy(esB_sb[:], esB_ps[:])
        nc.vector.tensor_mul(
            RB[:],
            esB_sb[:].unsqueeze(1).broadcast_to((128, 16, H)),
            indB16.unsqueeze(2).broadcast_to((128, 16, H)),
        )

        enb_t = psA.tile([128, NTB, H], F32, tag="en", name="enb")
        mm(enb_t[:], LBp, RB[:], start=True, stop=False,
           skip_group_check=True)
        for i in range(NTB):
            mm(enb_t[:, i, :], x1tp[:, TB * i:TB * i + 128], w0n4,
               start=False, stop=True, skip_group_check=True)
        pB = sm.tile([128, NTB, H], BF, tag="pB")
        lrB = sm.tile([128, NTB, H], F32, tag="lrB")
        nc.vector.tensor_scalar_mul(lrB[:], enb_t[:], LEAKY)
        nc.vector.tensor_max(lrB[:], lrB[:], enb_t[:])
        nc.scalar.activation(pB[:], lrB[:], AF.Exp)
        # group-sum reuses the logits PSUM region (logits dead after exp)
        mm(enb_t[:], e10xp, pB[:], start=True, stop=True,
           skip_group_check=True)
        rcB = sm.tile([128, NTB, H], F32, tag="rcB")
        nc.vector.reciprocal_approx_fast(rcB[:], enb_t[:])
        alB = sm.tile([128, NTB, H], BF, tag="alB")
        nc.vector.tensor_mul(alB[:], pB[:], rcB[:])
        albdB = sm.tile([128, NTB, 8, H], BF, tag="albdB")
        nc.vector.tensor_mul(
            albdB[:],
            alB[:].unsqueeze(2).broadcast_to((128, NTB, 8, H)),
            e10p.unsqueeze(1).unsqueeze(3).broadcast_to((128, NTB, 8, H)),
        )
        xb_ps = psb.tile([128, NTB, 8, H], F32, tag="agg")
        for i in range(NTB):
            mm(xb_ps[:, i, :, :], x1r[:, i, :], albdB[:, i, :, :],
               start=True, stop=True, skip_group_check=True)
        xb_bf = smb.tile([128, NTB, 8, H], BF, tag="xbbf")
        nc.vector.tensor_copy(xb_bf[:], xb_ps[:])
        hb_ps = psb.tile([128, H, BC], F32, tag="hn")
        for h in range(H):
            mm(hb_ps[:, h, :], w0b[:, 128 * h:128 * (h + 1)],
               xb_bf[:, :, :, h],
               start=True, stop=True, skip_group_check=True)
        nc.vector.tensor_copy(h0t_bf[:], hb_ps[:])

        # ---------- stage-C e_s (needs h0_new) ----------
        esC_ps = psb.tile([128, H], F32, tag="agg", name="esC_ps")
        for k in range(4):
            mm(esC_ps[:], h0t_bf[:, k, :], w1s4[:, k, :],
               start=(k == 0), stop=(k == 3), skip_group_check=True)
        esC_sb = smb.tile([128, H], BF, tag="esC")
        nc.vector.tensor_copy(esC_sb[:], esC_ps[:])
        nc.vector.tensor_mul(
            RC[:],
            esC_sb[:].unsqueeze(1).broadcast_to((128, 16, H)),
            indB16.unsqueeze(2).broadcast_to((128, 16, H)),
        )

        # ---------- STAGE A superiters ----------
        x2r_v = x2r.rearrange("(s p) tf -> s p tf", s=NSUP)
        x2t_v = x2t8.rearrange("p (s c) -> s p c", s=NSUP)
        for s in range(NSUP):
            # Chunk each stream across both HWDGE queues: one InstDMACopy's
            # descriptors land on only ~5 engines (packet-granular engine
            # assignment), so balance bytes between the two queue engine
            # sets with interleaved partition-chunks.
            x2r_t = stream.tile([TR, TPS, FEAT], BF, tag="x2r")
            x2r_f = x2r_t[:].rearrange("p a b -> p (a b)")
            for ci, (lo, hi) in enumerate(((0, 32), (32, 64), (64, 96),
                                           (96, TR))):
                eng = nc.sync if ci % 2 == 0 else nc.scalar
                eng.dma_start(x2r_f[lo:hi, :], x2r_v[s][lo:hi, :])
            x2t_t = stream.tile([FEAT, X2TP], FP8, tag="x2t")
            for ci, (lo, hi) in enumerate(((0, 32), (32, 64), (64, 96),
                                           (96, 128))):
                eng = nc.scalar if ci % 2 == 0 else nc.sync
                eng.dma_start(x2t_t[lo:hi, :], x2t_v[s][lo:hi, :])

            # logits: expander (e_s) first, then e_n accumulates.
            # ONE start=True mm for the whole tile: start marks pending-zero
            # at 2KB bank granularity, so two start=True writes into the
            # same bank lose the first one's data.
            en_t = psA.tile([128, TPS, H], F32, tag="en", name="en")
            mm(en_t[:], L80p, R_all[:, 2 * s:2 * s + 2, :, :],
               start=True, stop=False, skip_group_check=True)
            for t in range(TPS):
                mm(en_t[:, t, :], x2t_t[:, 125 * t:125 * t + 128], w0n4,
                   start=False, stop=True, skip_group_check=True)

            lr = sm.tile([128, TPS, H], F32, tag="lr")
            nc.vector.tensor_scalar_mul(lr[:], en_t[:], LEAKY)
            nc.vector.tensor_max(lr[:], lr[:], en_t[:])
            p = sm.tile([128, TPS, H], BF, tag="p")
            nc.scalar.activation(p[:], lr[:], AF.Exp)
            # group-sum reuses the logits PSUM region
            mm(en_t[:], e5xp, p[:], start=True, stop=True,
               skip_group_check=True)
            rc = sm.tile([128, TPS, H], F32, tag="rc")
            nc.vector.reciprocal_approx_fast(rc[:], en_t[:])
            al = sm.tile([128, TPS, H], BF, tag="al")
            nc.vector.tensor_mul(al[:], p[:], rc[:])
            albd = sm.tile([128, TPS, 5, H], BF, tag="albd")
            nc.vector.tensor_mul(
                albd[:],
                al[:].unsqueeze(2).broadcast_to((128, TPS, 5, H)),
                e5p.unsqueeze(1).unsqueeze(3).broadcast_to((128, TPS, 5, H)),
            )

            if debug_out and s == 0:
                dbg_lr = nc.dram_tensor("dbg_lr", [128, TPS * H], F32,
                                        kind="ExternalOutput").ap()
                nc.sync.dma_start(
                    dbg_lr.rearrange("p (a b) -> p a b", a=TPS), lr[:])
                dbg_al = nc.dram_tensor("dbg_al", [128, TPS * 5 * H], BF,
                                        kind="ExternalOutput").ap()
                nc.sync.dma_start(
                    dbg_al.rearrange("p (a b c) -> p a b c", a=TPS, b=5),
                    albd[:])

            # aggregation: x_agg^T[f, (t, g, h)]
            xa_bf = smb.tile([128, TPS, 5, H], BF, tag="xabf")
            for j in range(2):
                xa_ps = psb.tile([128, TPS // 2, 20], F32, tag="agg",
                                 name=f"xa{j}")
                for t2 in range(16):
                    t = 16 * j + t2
                    mm(xa_ps[:, t2, :], x2r_t[:, t, :],
                       albd[:TR, t, :, :], start=True, stop=True,
                       skip_group_check=True)
                nc.scalar.copy(
                    xa_bf[:, 16 * j:16 * (j + 1), :, :].rearrange(
                        "p a b c -> p (a b c)"),
                    xa_ps[:].rearrange("p t x -> p (t x)"))

            # h1_new^T = W0_h^T @ x_agg_h
            for j in range(2):
                hn_ps = psb.tile([128, 2, SUPG], F32, tag="hn",
                                 name=f"hnps{j}")
                for h2_ in range(2):
                    h = 2 * j + h2_
                    mm(hn_ps[:, h2_, :], w0b[:, 128 * h:128 * (h + 1)],
                       xa_bf[:, :, :, h],
                       start=True, stop=True, skip_group_check=True)
                nc.vector.tensor_copy(
                    h1t_s[:, 2 * j:2 * (j + 1), SUPG * s:SUPG * (s + 1)],
                    hn_ps[:])

            # transpose h1_new slice to row-major (2 tiles of 80 groups)
            for i2 in range(2):
                i = 2 * s + i2
                tr_ps = psb.tile([128, H, 128], F32, tag="hn", name="tr_ps")
                for h in range(H):
                    mm(tr_ps[:, h, :], h1t_s[:, h, TB * i:TB * i + 128],
                       i128b, start=True, stop=True, skip_group_check=True)
                nc.scalar.copy(h1r_s[:, i, :, :], tr_ps[:TB, :, :])

            # ---- stage C (layer 1) for this superiter's two tiles ----
            encp = psC.tile([128, 2, H], F32, tag="cen", name="encp")
            mm(encp[:], LBp, RC[:, 2 * s:2 * s + 2, :], start=True,
               stop=False, skip_group_check=True)
            for i2 in range(2):
                i = 2 * s + i2
                for k in range(4):
                    mm(encp[:, i2, :], h1t_s[:, k, TB * i:TB * i + 128],
                       w1n4[:, k, :], start=False, stop=(k == 3),
                       skip_group_check=True)
            lrc = sm.tile([128, 2, H], F32, tag="lrc")
            nc.vector.tensor_scalar_mul(lrc[:], encp[:], LEAKY)
            nc.vector.tensor_max(lrc[:], lrc[:], encp[:])
            pc = sm.tile([128, 2, H], BF, tag="pc")
            nc.scalar.activation(pc[:], lrc[:], AF.Exp)
            # group-sum reuses the stage-C logits PSUM region
            mm(encp[:], e10xp, pc[:], start=True, stop=True,
               skip_group_check=True)
            rcc = sm.tile([128, 2, H], F32, tag="rcc")
            nc.vector.reciprocal_approx_fast(rcc[:], encp[:])
            alc = sm.tile([128, 2, H], BF, tag="alc")
            nc.vector.tensor_mul(alc[:], pc[:], rcc[:])
            albdc = sm.tile([128, 2, 8, H], BF, tag="albdc")
            nc.vector.tensor_mul(
                albdc[:],
                alc[:].unsqueeze(2).broadcast_to((128, 2, 8, H)),
                e10p.unsqueeze(1).unsqueeze(3).broadcast_to((128, 2, 8, H)),
            )
            xc_ps = psC.tile([128, 2, 4, 8, H], F32, tag="cagg", name="xc_ps")
            for i2 in range(2):
                i = 2 * s + i2
                for k in range(4):
                    mm(xc_ps[:, i2, k, :, :], h1r_s[:, i, k, :],
                       albdc[:TB, i2, :, :], start=True, stop=True,
                       skip_group_check=True)
            nc.vector.tensor_copy(
                xc_sb[:, :, 2 * s:2 * s + 2, :, :].transpose([0, 2, 1, 3, 4]),
                xc_ps[:])

        # ---------- stage C epilogue: h0_fin = x_aggC @ W1 heads ----------
        hf_ps = psb.tile([128, H, BC], F32, tag="hn")
        for h in range(H):
            for k in range(4):
                mm(hf_ps[:, h, :], w1b[:, k, 128 * h:128 * (h + 1)],
                   xc_sb[:, k, :, :, h],
                   start=(k == 0), stop=(k == 3), skip_group_check=True)
        hf_bf = smb.tile([128, H, BC], BF, tag="hfbf")
        nc.scalar.copy(hf_bf[:], hf_ps[:])

        # ---------- FC + output transpose ----------
        of_ps = psb.tile([128, 2, BC], F32, tag="agg")
        for m in range(2):
            for k in range(4):
                mm(of_ps[:, m, :], wfcb[:, k, 128 * m:128 * (m + 1)],
                   hf_bf[:, k, :], start=(k == 0), stop=(k == 3),
                   skip_group_check=True)
        ot_bf = smb.tile([128, 2, BC], BF, tag="otbf")
        nc.vector.tensor_copy(ot_bf[:], of_ps[:])
        or_ps = psb.tile([BC, 2, 128], F32, tag="hn", name="or_ps")
        for m in range(2):
            mm(or_ps[:, m, :], ot_bf[:, m, :], i128b, start=True, stop=True,
               skip_group_check=True)
        or_sb = smb.tile([BC, 2, 128], F32, tag="orsb")
        nc.vector.tensor_copy(or_sb[:], or_ps[:])
        nc.sync.dma_start(out_d.rearrange("b (m o) -> b m o", m=2), or_sb[:])

        if debug_out:
            dbg_h1t = nc.dram_tensor("dbg_h1t", [128, H * G1P], BF,
                                     kind="ExternalOutput").ap()
            dbg_h0t = nc.dram_tensor("dbg_h0t", [128, H * BC], BF,
                                     kind="ExternalOutput").ap()
            dbg_xc = nc.dram_tensor("dbg_xc", [128, 4 * NTB * 8 * H], BF,
                                    kind="ExternalOutput").ap()
            nc.sync.dma_start(
                dbg_h1t.rearrange("p (a b) -> p a b", a=H), h1t_s[:])
            nc.sync.dma_start(
                dbg_h0t.rearrange("p (a b) -> p a b", a=H), h0t_bf[:])
            nc.sync.dma_start(
                dbg_xc.rearrange("p (a b c d) -> p a b c d", a=4, b=NTB, c=8),
                xc_sb[:])

    nc.compile()
    return nc


def _host_prep(h0, h1, h2, W0, a0_s, a0_n, W1, a1_s, a1_n, W_fc):
    bf16 = ml_dtypes.bfloat16
    fp8 = ml_dtypes.float8_e4m3
    f32 = np.float32

    def combo(W, a):  # [F, H*D], [H, D] -> [F, H]
        F_ = W.shape[0]
        return np.einsum("fhd,hd->fh", W.reshape(F_, H, 128), a).astype(f32)

    w0s = combo(W0, a0_s)
    w0n = combo(W0, a0_n)
    w1s = combo(W1, a1_s).reshape(4, 128, H).transpose(1, 0, 2)   # [128,4,H]
    w1n = combo(W1, a1_n).reshape(4, 128, H).transpose(1, 0, 2)

    ar = np.arange
    sm_shared = {}

    def put(name, arr):
        a = np.zeros((128, dict(SMALLS)[name]), dtype=bf16)
        a[:arr.shape[0], :arr.shape[1]] = arr.astype(bf16)
        sm_shared[name] = a

    put("w0s4", w0s)
    put("w0n4", w0n)
    put("w1s4", w1s.reshape(128, 4 * H))
    put("w1n4", w1n.reshape(128, 4 * H))
    put("w0b", W0.astype(f32))
    put("w1b", W1.reshape(4, 128, HID).transpose(1, 0, 2).reshape(128, -1))
    put("wfcb", W_fc.reshape(4, 128, OUT).transpose(1, 0, 2).reshape(128, -1))
    e5p = (ar(128)[:, None] // 25 == ar(5)[None, :]) & (ar(128)[:, None] < 125)
    put("e5p", e5p.astype(f32))
    e5x = np.zeros((128, 128), dtype=f32)
    blk = (ar(125)[:, None] // 25 == ar(125)[None, :] // 25)
    e5x[:125, :125] = blk
    e5x[125:, :] = 0.0
    for m in range(125, 128):
        e5x[m, m] = 1.0
    put("e5xp", e5x)
    L80 = np.zeros((128, 128), dtype=f32)
    L80[:80, :125] = (ar(80)[:, None] % 5 == ar(125)[None, :] // 25)
    put("L80p", L80)
    LB = np.zeros((128, 128), dtype=f32)
    LB[:, :80] = (ar(128)[:, None] % 8 == ar(80)[None, :] // 10)
    put("LBp", LB)
    ind16 = np.zeros((128, 16), dtype=f32)
    ind16[:80] = (ar(80)[:, None] // 5 == ar(16)[None, :])
    put("ind16", ind16)
    put("indB16", (ar(128)[:, None] // 8 == ar(16)[None, :]).astype(f32))
    put("i128b", np.eye(128, dtype=f32))
    e10 = np.zeros((128, 8), dtype=f32)
    e10[:80] = (ar(80)[:, None] // 10 == ar(8)[None, :])
    put("e10p", e10)
    e10x = np.zeros((128, 128), dtype=f32)
    e10x[:80, :80] = (ar(80)[:, None] // 10 == ar(80)[None, :] // 10)
    for m in range(80, 128):
        e10x[m, m] = 1.0
    put("e10xp", e10x)

    in_maps = []
    for c in range(NCORES):
        sl = slice(c * BC, (c + 1) * BC)
        h2c = np.asarray(h2[sl], dtype=f32).reshape(R2, FEAT)
        h1c = np.asarray(h1[sl], dtype=f32).reshape(G1, FEAT)
        h0c = np.asarray(h0[sl], dtype=f32)
        m = dict(sm_shared)
        x1tp = np.zeros((128, G1P), dtype=bf16)
        x1tp[:, :G1] = h1c.T.astype(bf16)
        m["x1tp"] = x1tp
        m["x0t"] = np.zeros((128, BC), dtype=bf16)
        m["x0t"][:] = h0c.T.astype(bf16)
        x1r = np.zeros((128, NTB, FEAT), dtype=bf16)
        x1r[:TB] = h1c.astype(bf16).reshape(NTB, TB, FEAT).transpose(1, 0, 2)
        m["x1r"] = x1r.reshape(128, NTB * FEAT)
        # pack the smalls in layout order
        packed = np.concatenate([m.pop(n) for n, _ in SMALLS], axis=1)
        mm_ = {"smalls": np.ascontiguousarray(packed)}
        mm_["x2r"] = np.ascontiguousarray(
            h2c.astype(bf16).reshape(NSUP, TPS, TR, FEAT)
            .transpose(0, 2, 1, 3).reshape(NSUP * TR, TPS * FEAT))
        x2t = np.zeros((FEAT, NSUP, X2TP), dtype=fp8)
        x2t[:, :, :SUPR] = h2c.T.astype(fp8).reshape(FEAT, NSUP, SUPR)
        mm_["x2t8"] = np.ascontiguousarray(x2t.reshape(FEAT, NSUP * X2TP))
        in_maps.append(mm_)
    return in_maps


_PROGRAM = None


def kernel(**inputs):
    global _PROGRAM
    if _PROGRAM is None:
        _PROGRAM = build_program()
    in_maps = _host_prep(**{k: np.asarray(v) for k, v in inputs.items()})
    res = bass_utils.run_bass_kernel_spmd(
        _PROGRAM, in_maps, core_ids=list(range(NCORES)))
    return np.concatenate([r["out"] for r in res.results], axis=0)


if __name__ == "__main__":
    build_program()
    print("program built + compiled OK")
